# revision 6
# baseline (speedup 1.0000x reference)
"""Trainium2 Bass kernel for DiffuserAttention — dense-adjacency design.

v4: replaces the per-edge dma_gather pipeline (SWDGE desc-gen bound at
~6-9ns/row on the single Pool engine) with a dense attention matrix:
A[h][src, dst_local] = multiplicity * exp(k_src . q_dst), built once on
device (dense scores via PE, exp on ACT, structural multiplicity mask
M from host via DVE), stored fp8 in DRAM (100MB/core).  Each diffusion
step is then a pure dense stream: for each head and 128-src chunk,
psum[feat, dst] += h_chunk[src, feat]^T-style matmul with A as the
moving operand — contiguous HBM reads at full bandwidth, zero DMA
descriptq generation.  Softmax denominators come free from a ones
column appended to the step-1 stationary (row 64 of psum = column sums
of A); normalization (0.9/denom) and the 0.1*v teleport are applied
per-dst on DVE after each head's accumulation.  h lives in SBUF as
[128 src-part, chunk, head, 65] fp8 and is refreshed per step by a
per-head-batch AllGather (4 column-sliced collectives per step so the
first batches' tables land while later batches still compute).
"""

import sys

sys.path.insert(0, "/opt/trn_rl_repo")

import math

import numpy as np
import ml_dtypes

import concourse.bass as bass
import concourse.bacc as bacc
import concourse.mybir as mybir
import concourse.tile as tile
from concourse.bass_utils import run_bass_kernel_spmd

F32 = mybir.dt.float32
BF16 = mybir.dt.bfloat16
FP8 = mybir.dt.float8e4
NPBF16 = ml_dtypes.bfloat16
NPFP8 = ml_dtypes.float8_e4m3

NCORES = 8
ALPHA = 0.1
NSTEPS = 5
LN_EPS = 1e-12

B, S, D, H = 2, 4096, 768, 12
HD = D // H          # 64
N = B * S            # 8192
NR = N // NCORES     # 1024 dst rows per core
SC = N // 128        # 64 src chunks
HP = H // 2          # 6 head pairs
HB = 4               # head batches
HBH = H // HB        # 3 heads per batch
NTO = NR // 128      # 8
DC = D // 128        # 6
SCG = SC // 8        # src chunk groups (= ranks)


def host_prep(hidden_states, attention_mask, src, dst,
              Wq, bq, Wk, bk, Wv, bv, Wo, bo, ln_g, ln_b):
    x = np.asarray(hidden_states, np.float32).reshape(N, D)
    src = np.asarray(src).astype(np.int64)
    dst = np.asarray(dst).astype(np.int64)
    mask1 = np.asarray(attention_mask).reshape(-1) >= 0

    scale_q = 1.0 / math.sqrt(HD)
    Wq_s = np.ascontiguousarray((np.asarray(Wq) * scale_q).astype(NPBF16))
    Wk_s = np.ascontiguousarray(np.asarray(Wk).astype(NPBF16))
    Wv_s = np.ascontiguousarray(np.asarray(Wv).astype(NPBF16))
    # per-feature bias rows for the transposed projections: [128, 3, DC]
    biasT = np.zeros((128, 3, DC), np.float32)
    for wi, bvec in enumerate([np.asarray(bq) * scale_q, bk, bv]):
        biasT[:, wi, :] = np.asarray(bvec, np.float32).reshape(DC, 128).T
    bias_v = np.ascontiguousarray(
        np.broadcast_to(np.asarray(bv, np.float32), (128, D)))
    zero_bias = not (np.any(biasT) or np.any(bias_v))

    Wo_h = np.ascontiguousarray(
        np.asarray(Wo, np.float32).reshape(H, HD, D).transpose(1, 0, 2)
        .astype(NPBF16))                                   # [64, H, D]
    ident = np.eye(128, dtype=np.float32).astype(NPBF16)
    g_rep = np.ascontiguousarray(
        np.broadcast_to(np.asarray(ln_g, np.float32), (128, D)))
    b_rep = np.ascontiguousarray(
        np.broadcast_to(np.asarray(ln_b, np.float32), (128, D)))

    valid = mask1[src] & mask1[dst]
    in_maps = []
    for c in range(NCORES):
        rows = slice(c * NR, (c + 1) * NR)
        xTown = np.ascontiguousarray(x[rows].T.astype(NPBF16))
        xb = np.ascontiguousarray(x[rows] + np.asarray(bo, np.float32))
        # structural multiplicity mask for this core's dst range
        sel = np.nonzero((dst >= c * NR) & (dst < (c + 1) * NR) & valid)[0]
        Mcnt = np.zeros(N * NR, np.float32)
        np.add.at(Mcnt, src[sel] * NR + (dst[sel] - c * NR), 1.0)
        M_in = np.ascontiguousarray(
            Mcnt.reshape(SC, 128, NR).astype(NPFP8))
        in_maps.append(dict(
            xTown=xTown, Wq=Wq_s, Wk=Wk_s, Wv=Wv_s, biasT=biasT,
            bias_v=bias_v, Wo_h=Wo_h, xb=xb, g_rep=g_rep, b_rep=b_rep,
            ident=ident, M_in=M_in,
        ))
    return in_maps, zero_bias


def build_program(zero_bias=False):
    nc = bacc.Bacc(None, target_bir_lowering=False, debug=False,
                   num_devices=NCORES)

    xTown_in = nc.dram_tensor("xTown", [D, NR], BF16, kind="ExternalInput")
    Wq_in = nc.dram_tensor("Wq", [D, D], BF16, kind="ExternalInput")
    Wk_in = nc.dram_tensor("Wk", [D, D], BF16, kind="ExternalInput")
    Wv_in = nc.dram_tensor("Wv", [D, D], BF16, kind="ExternalInput")
    biasT_in = nc.dram_tensor("biasT", [128, 3, DC], F32, kind="ExternalInput")
    bias_v_in = nc.dram_tensor("bias_v", [128, D], F32, kind="ExternalInput")
    Wo_in = nc.dram_tensor("Wo_h", [HD, H, D], BF16, kind="ExternalInput")
    xb_in = nc.dram_tensor("xb", [NR, D], F32, kind="ExternalInput")
    g_rep_in = nc.dram_tensor("g_rep", [128, D], F32, kind="ExternalInput")
    b_rep_in = nc.dram_tensor("b_rep", [128, D], F32, kind="ExternalInput")
    ident_in = nc.dram_tensor("ident", [128, 128], BF16, kind="ExternalInput")
    M_in = nc.dram_tensor("M_in", [SC, 128, NR], FP8, kind="ExternalInput")

    out_ext = nc.dram_tensor("out", [NR, D], F32, kind="ExternalOutput")

    kTloc = nc.dram_tensor("kTloc", [D, NR], BF16, kind="Internal")
    vshard = nc.dram_tensor("vshard", [NR, D], FP8, kind="Internal")
    A_dram = nc.dram_tensor("A_dram", [HB, SCG, 2, 128, 4 * HBH, NR], FP8,
                            kind="Internal")
    shard_hb = [nc.dram_tensor(f"shard{b_}", [NR, HBH * HD], FP8,
                               kind="Internal") for b_ in range(HB)]

    kTall = nc.dram_tensor("kTall", [NCORES * D, NR], BF16, kind="Internal",
                           addr_space="Shared")
    tbl = nc.dram_tensor("tblsh", [N, D], FP8, kind="Internal",
                         addr_space="Shared")
    tbl_hb = [nc.dram_tensor(f"tbl{b_}", [N, HBH * HD], FP8, kind="Internal",
                             addr_space="Shared") for b_ in range(HB)]

    AG = [list(range(NCORES))]

    with tile.TileContext(nc) as tc:
        with (
            tc.tile_pool(name="res", bufs=1) as res,
            tc.tile_pool(name="span", bufs=1) as span,
        ):
            ident_sb = res.tile([128, 128], BF16)
            nc.sync.dma_start(ident_sb[:], ident_in[:])

            # persistent across phases
            qT_sb = span.tile([128, HP, NR], BF16)      # q^T, 2 heads/row
            v01T_sb = span.tile([HD, H, NR], FP8)       # 0.1*v^T per head
            h5T_sb = span.tile([HD, H, NR], BF16)       # step-5 output
            rdenomB = span.tile([HD, H, NR], FP8)       # 0.9/denom per head

            # =========== P0: projections ===========
            with (
                tc.tile_pool(name="p0c", bufs=1) as p0c,
                tc.tile_pool(name="p0", bufs=3) as p0,
                tc.tile_pool(name="p0ps", bufs=2, space="PSUM") as p0ps,
                tc.tile_pool(name="p0ps2", bufs=2, space="PSUM") as p0ps2,
            ):
                xTo_sb = p0c.tile([128, DC, NR], BF16)
                nc.sync.dma_start(
                    xTo_sb[:], xTown_in[:].rearrange("(c p) n -> p c n", p=128))
                W_sb = p0c.tile([128, 3, DC, D], BF16)
                for i, W in enumerate([Wq_in, Wk_in, Wv_in]):
                    nc.sync.dma_start(
                        W_sb[:, i, :, :],
                        W[:].rearrange("(c p) g -> p c g", p=128))
                if not zero_bias:
                    biasT_sb = p0c.tile([128, 3, DC], F32)
                    nc.sync.dma_start(biasT_sb[:], biasT_in[:])
                    bias_v_sb = p0c.tile([128, D], F32)
                    nc.sync.dma_start(bias_v_sb[:], bias_v_in[:])

                # v node-major -> vshard (fp8) -> AllGather => h0 table
                for t in range(NTO):
                    ps = p0ps.tile([128, D], F32, tag="vps")
                    for c in range(DC):
                        for j in range(2):
                            js = slice(j * 512, min((j + 1) * 512, D))
                            nc.tensor.matmul(
                                ps[:, js], xTo_sb[:, c, t * 128:(t + 1) * 128],
                                W_sb[:, 2, c, js],
                                start=(c == 0), stop=(c == DC - 1))
                    stg8 = p0.tile([128, D], FP8, tag="vstg")
                    if zero_bias:
                        nc.scalar.copy(stg8[:], ps[:])
                    else:
                        nc.vector.tensor_tensor(
                            stg8[:], ps[:], bias_v_sb[:], mybir.AluOpType.add)
                    nc.sync.dma_start(vshard[t * 128:(t + 1) * 128, :], stg8[:])
                nc.gpsimd.collective_compute(
                    "AllGather", mybir.AluOpType.bypass, replica_groups=AG,
                    ins=[vshard[:].opt()], outs=[tbl[:].opt()])

                # transposed projections: out[f, tok] per feature chunk
                # (k first so its AllGather overlaps the q/v projections)
                for wi in (1, 0, 2):
                    for fc in range(DC):
                        pst = p0ps2.tile([128, NR], F32, tag="tps")
                        for dc in range(DC):
                            for j in range(2):
                                js = slice(j * 512, (j + 1) * 512)
                                nc.tensor.matmul(
                                    pst[:, js],
                                    W_sb[:, wi, dc, fc * 128:(fc + 1) * 128],
                                    xTo_sb[:, dc, js],
                                    start=(dc == 0), stop=(dc == DC - 1))
                        if not zero_bias:
                            nc.vector.scalar_tensor_tensor(
                                pst[:], pst[:], biasT_sb[:, wi, fc:fc + 1],
                                pst[:], mybir.AluOpType.add,
                                mybir.AluOpType.bypass)
                        if wi == 0:
                            nc.vector.tensor_copy(qT_sb[:, fc, :], pst[:])
                        elif wi == 1:
                            stg = p0.tile([128, NR], BF16, tag="kstg")
                            nc.vector.tensor_copy(stg[:], pst[:])
                            nc.sync.dma_start(
                                kTloc[fc * 128:(fc + 1) * 128, :], stg[:])
                        else:
                            t1 = p0.tile([128, NR], FP8, tag="v1")
                            nc.vector.tensor_scalar(
                                t1[:], pst[:], ALPHA, None,
                                mybir.AluOpType.mult)
                            # split head pair rows to per-head base-0 layout
                            nc.sync.dma_start(
                                v01T_sb[:, 2 * fc, :], t1[0:HD, :])
                            nc.sync.dma_start(
                                v01T_sb[:, 2 * fc + 1, :], t1[HD:128, :])
                    if wi == 1:
                        nc.gpsimd.collective_compute(
                            "AllGather", mybir.AluOpType.bypass,
                            replica_groups=AG,
                            ins=[kTloc[:].opt()], outs=[kTall[:].opt()])

            # =========== P1: dense A build ===========
            with (
                tc.tile_pool(name="p1c", bufs=1) as p1c,
                tc.tile_pool(name="p1m", bufs=2) as p1m,
                tc.tile_pool(name="p1s", bufs=3) as p1s,
                tc.tile_pool(name="p1a", bufs=2) as p1a,
                tc.tile_pool(name="p1ps", bufs=3, space="PSUM") as p1ps,
            ):
                kT_sb = p1c.tile([128, HP, N], BF16)
                for r in range(NCORES):
                    for fc in range(DC):
                        nc.sync.dma_start(
                            kT_sb[:, fc, r * NR:(r + 1) * NR],
                            kTall[r * D + fc * 128:r * D + (fc + 1) * 128, :])
                for sc in range(SC):
                    Mt = p1m.tile([128, NR], FP8, tag="Mt")
                    nc.sync.dma_start(Mt[:], M_in[sc, :, :])
                    for h in range(H):
                        o = (h % 2) * HD
                        hp = h // 2
                        ps = p1ps.tile([128, NR], F32, tag="sps")
                        for j in range(2):
                            js = slice(j * 512, (j + 1) * 512)
                            nc.tensor.matmul(
                                ps[:, js],
                                kT_sb[o:o + HD, hp, sc * 128:(sc + 1) * 128],
                                qT_sb[o:o + HD, hp, js],
                                start=True, stop=True)
                        stage = p1s.tile([128, NR], BF16, tag="stage")
                        nc.scalar.activation(
                            stage[:], ps[:],
                            mybir.ActivationFunctionType.Exp)
                        j3 = h % HBH
                        if j3 == 0:
                            Ast = p1a.tile([128, HBH, NR], FP8, tag="Ast")
                        nc.vector.tensor_tensor(
                            Ast[:, j3, :], stage[:], Mt[:],
                            mybir.AluOpType.mult)
                        if j3 == HBH - 1:
                            s4 = (sc % 4) * HBH
                            nc.sync.dma_start(
                                A_dram[h // HBH, sc // 8, (sc % 8) // 4,
                                       :, s4:s4 + HBH, :],
                                Ast[:])

            # =========== P2: diffusion steps ===========
            with (
                tc.tile_pool(name="p2c", bufs=1) as p2c,
                tc.tile_pool(name="p2a", bufs=5) as p2a,
                tc.tile_pool(name="p2t", bufs=2) as p2t,
                tc.tile_pool(name="p2s1", bufs=1) as p2s1,
                tc.tile_pool(name="p2sh", bufs=2) as p2sh,
                tc.tile_pool(name="p2rs", bufs=1) as p2rs,
                tc.tile_pool(name="p2ps", bufs=1, space="PSUM") as p2ps,
                tc.tile_pool(name="p2tp", bufs=2, space="PSUM") as p2tp,
            ):
                h_sb = p2c.tile([128, SC, H, 68], FP8)
                nc.vector.memset(h_sb[:, :, :, 64:65], 1.0)
                for hb0 in range(HB):
                    rstg = p2rs.tile([128, SC, HBH * HD], FP8, tag="rstg")
                    nc.scalar.dma_start(
                        rstg[:],
                        tbl[:, hb0 * HBH * HD:(hb0 + 1) * HBH * HD]
                        .rearrange("(c p) f -> p c f", p=128))
                    nc.scalar.copy(
                        h_sb[:, :, hb0 * HBH:(hb0 + 1) * HBH, 0:64],
                        rstg[:].rearrange("p c (h f) -> p c h f", h=HBH))

                for step in range(1, NSTEPS + 1):
                    ones = 1 if step == 1 else 0
                    for hb in range(HB):
                        heads = [hb * HBH + j for j in range(HBH)]
                        psl = []
                        for j in range(HBH):
                            accj = p2ps.tile([64 + ones, NR], F32,
                                             tag=f"acc{j}")
                            psl.append(accj)
                        for scg in range(SCG):
                            for half in range(2):
                                Asup = p2a.tile([128, 4 * HBH, NR], FP8,
                                                tag="Asup")
                                nc.sync.dma_start(
                                    Asup[:], A_dram[hb, scg, half])
                                Ar = Asup[:].rearrange(
                                    "p (s g) d -> p s g d", g=HBH)
                                for p4 in range(2):
                                    sc0 = scg * 8 + half * 4 + 2 * p4
                                    for j, h in enumerate(heads):
                                        stat = h_sb[:, sc0:sc0 + 2, h,
                                                    0:64 + ones]
                                        for jj in range(2):
                                            js = slice(jj * 512,
                                                       (jj + 1) * 512)
                                            nc.tensor.matmul(
                                                psl[j][:, js], stat,
                                                Ar[:, 2 * p4:2 * p4 + 2,
                                                   j, js],
                                                start=(sc0 == 0),
                                                stop=(sc0 == SC - 2),
                                                perf_mode=mybir
                                                .MatmulPerfMode.DoubleRow)
                        shard_st = None
                        if step < NSTEPS:
                            shard_st = p2sh.tile([128, NTO, HBH * HD], FP8,
                                                 tag="shst")
                        for j, h in enumerate(heads):
                            if step == 1:
                                den65 = p2s1.tile([65, NR], F32, tag="den65")
                                nc.vector.tensor_copy(
                                    den65[64:65, :], psl[j][64:65, :])
                                dnr = p2s1.tile([1, NR], F32, tag="dnr")
                                nc.sync.dma_start(dnr[:], den65[64:65, :])
                                nc.vector.tensor_scalar(
                                    dnr[:], dnr[:], 1e-9, None,
                                    mybir.AluOpType.max)
                                rc = p2s1.tile([1, NR], F32, tag="rc")
                                nc.vector.reciprocal(rc[:], dnr[:])
                                rc8 = p2s1.tile([1, NR], FP8, tag="rc8")
                                nc.vector.tensor_scalar(
                                    rc8[:], rc[:], 64.0 * (1.0 - ALPHA),
                                    None, mybir.AluOpType.mult)
                                nc.gpsimd.partition_broadcast(
                                    rdenomB[:, h, :], rc8[0:1, :],
                                    channels=HD)
                            t1 = p2t.tile([HD, NR], BF16, tag="t1")
                            nc.vector.tensor_tensor(
                                t1[:], psl[j][0:HD, :], rdenomB[:, h, :],
                                mybir.AluOpType.mult)
                            if step < NSTEPS:
                                hT = p2t.tile([HD, NR], BF16, tag="hT")
                                nc.vector.scalar_tensor_tensor(
                                    hT[:], t1[:], 1.0 / 64.0,
                                    v01T_sb[:, h, :],
                                    mybir.AluOpType.mult,
                                    mybir.AluOpType.add)
                                for tb in range(NTO):
                                    tp = p2tp.tile([128, HD], BF16, tag="tp")
                                    nc.tensor.transpose(
                                        tp[:],
                                        hT[:, tb * 128:(tb + 1) * 128],
                                        ident_sb[0:HD, 0:HD])
                                    nc.scalar.copy(
                                        shard_st[:, tb,
                                                 j * HD:(j + 1) * HD],
                                        tp[:])
                            else:
                                nc.vector.scalar_tensor_tensor(
                                    h5T_sb[:, h, :], t1[:], 1.0 / 64.0,
                                    v01T_sb[:, h, :],
                                    mybir.AluOpType.mult,
                                    mybir.AluOpType.add)
                        if step < NSTEPS:
                            nc.scalar.dma_start(
                                shard_hb[hb][:].rearrange(
                                    "(t p) f -> p t f", p=128),
                                shard_st[:])
                            nc.gpsimd.collective_compute(
                                "AllGather", mybir.AluOpType.bypass,
                                replica_groups=AG,
                                ins=[shard_hb[hb][:].opt()],
                                outs=[tbl_hb[hb][:].opt()])
                            rstg = p2rs.tile([128, SC, HBH * HD], FP8,
                                             tag="rstg")
                            nc.scalar.dma_start(
                                rstg[:],
                                tbl_hb[hb][:].rearrange(
                                    "(c p) f -> p c f", p=128))
                            nc.scalar.copy(
                                h_sb[:, :, hb * HBH:(hb + 1) * HBH, 0:64],
                                rstg[:].rearrange(
                                    "p c (h f) -> p c h f", h=HBH))

            # =========== P3: output projection + LN ===========
            with (
                tc.tile_pool(name="p3c", bufs=1) as p3c,
                tc.tile_pool(name="p3", bufs=2) as p3,
                tc.tile_pool(name="p3ps", bufs=2, space="PSUM") as p3ps,
            ):
                Wo_sb = p3c.tile([HD, H, D], BF16)
                nc.sync.dma_start(Wo_sb[:], Wo_in[:])
                g_sb = p3c.tile([128, D], F32)
                nc.sync.dma_start(g_sb[:], g_rep_in[:])
                b_sb = p3c.tile([128, D], F32)
                nc.sync.dma_start(b_sb[:], b_rep_in[:])
                for t in range(NTO):
                    yps = p3ps.tile([128, D], F32, tag="yps")
                    for h in range(H):
                        for j in range(2):
                            js = slice(j * 512, min((j + 1) * 512, D))
                            nc.tensor.matmul(
                                yps[:, js],
                                h5T_sb[:, h, t * 128:(t + 1) * 128],
                                Wo_sb[:, h, js],
                                start=(h == 0), stop=(h == H - 1))
                    xb_sb = p3.tile([128, D], F32, tag="xb")
                    nc.sync.dma_start(
                        xb_sb[:], xb_in[t * 128:(t + 1) * 128, :])
                    y_sb = p3.tile([128, D], F32, tag="y")
                    nc.vector.tensor_tensor(
                        y_sb[:], yps[:], xb_sb[:], mybir.AluOpType.add)
                    mu = p3.tile([128, 1], F32, tag="mu")
                    nc.vector.tensor_reduce(
                        mu[:], y_sb[:], mybir.AxisListType.X,
                        mybir.AluOpType.add)
                    negmu = p3.tile([128, 1], F32, tag="negmu")
                    nc.vector.tensor_scalar(
                        negmu[:], mu[:], -1.0 / D, None,
                        mybir.AluOpType.mult)
                    sq = p3.tile([128, D], F32, tag="sq")
                    var = p3.tile([128, 1], F32, tag="var")
                    nc.scalar.activation(
                        sq[:], y_sb[:], mybir.ActivationFunctionType.Square,
                        bias=negmu[:], scale=1.0, accum_out=var[:])
                    vs = p3.tile([128, 1], F32, tag="vs")
                    nc.vector.tensor_scalar(
                        vs[:], var[:], 1.0 / D, LN_EPS,
                        mybir.AluOpType.mult, mybir.AluOpType.add)
                    std = p3.tile([128, 1], F32, tag="std")
                    nc.scalar.sqrt(std[:], vs[:])
                    rstd = p3.tile([128, 1], F32, tag="rstd")
                    nc.vector.reciprocal(rstd[:], std[:])
                    t1 = p3.tile([128, D], F32, tag="t1")
                    nc.vector.scalar_tensor_tensor(
                        t1[:], y_sb[:], negmu[:], g_sb[:],
                        mybir.AluOpType.add, mybir.AluOpType.mult)
                    outt = p3.tile([128, D], F32, tag="outt")
                    nc.vector.scalar_tensor_tensor(
                        outt[:], t1[:], rstd[:], b_sb[:],
                        mybir.AluOpType.mult, mybir.AluOpType.add)
                    nc.sync.dma_start(
                        out_ext[t * 128:(t + 1) * 128, :], outt[:])

    nc.compile()
    return nc


_PROG_CACHE = {}


def _get_program(zero_bias):
    if zero_bias not in _PROG_CACHE:
        _PROG_CACHE[zero_bias] = build_program(zero_bias)
    return _PROG_CACHE[zero_bias]


def run(cfg, inputs, trace=False):
    in_maps, zero_bias = host_prep(**inputs)
    nc = _get_program(zero_bias)
    res = run_bass_kernel_spmd(
        nc, in_maps, core_ids=list(range(NCORES)), trace=trace)
    full = np.empty((N, D), np.float32)
    for r in range(NCORES):
        full[r * NR:(r + 1) * NR] = res.results[r]["out"]
    return full.reshape(B, S, D), res


def _cfg(**kw):
    return dict(kw)


def kernel(**inputs):
    out, _ = run(None, inputs)
    return out
